# revision 6
# baseline (speedup 1.0000x reference)
"""AO layer kernel for Trainium2 (8 NeuronCores, data-parallel over walkers).

Math: out[b,n,a] = ang(a, r) * rad(a, r),  r = pos[b,n] - centers[a]
  rad = sum_p coeffs[a,p] * exp(-exps[a,p] * |r|^2)
  ang = prod_c r_c^powers[a,c],  powers in {0,1,2}

Device formulation (per core, i = flattened (b,n) walker-electron index):
  Basis R[7, i] = {x^2, y^2, z^2, x, y, z, 1} of pos (built on device).
  z[(a,p), i]   = W1[7,(a,p)]^T R          (PE matmul; -alpha*r2 + ln|c| folded)
  E = exp(z)                               (ScalarE)
  rad[a, i]     = S[(a,p),a]^T E           (PE matmul with +-1 sign matrix)
  p_c[a, i]     = Q_c[7,a]^T R             (PE matmuls; per-axis angular polys)
  out[a, i]     = px*py*pz*rad             (VectorE)
  out[i, a]     = PE transpose, DMA to DRAM.
"""

import numpy as np

B, NEL, A, P = 512, 32, 256, 6
NCORES = 8
BS = B // NCORES          # 64 walkers per core
I = BS * NEL              # 2048 (b,n) pairs per core
ITILE = 512
NIT = I // ITILE          # 4 i-tiles
RT = (A * P) // 128       # 12 r-tiles of 128 (a,p) rows
K7 = 7

_CACHE = {}


def _build_nc():
    import concourse.bass as bass
    import concourse.bacc as bacc
    import concourse.tile as tile
    import concourse.mybir as mybir
    from concourse import masks

    f32 = mybir.dt.float32
    f32r = mybir.dt.float32r
    EXP = mybir.ActivationFunctionType.Exp
    PSUM = bass.MemorySpace.PSUM

    nc = bacc.Bacc("TRN2", target_bir_lowering=False, debug=False,
                   num_devices=NCORES)

    pos_d = nc.declare_dram_parameter("pos", [I, 3], f32, isOutput=False)
    w1_d = nc.declare_dram_parameter("w1", [K7, RT * 128], f32, isOutput=False)
    s_d = nc.declare_dram_parameter("s", [128, RT * 128], f32, isOutput=False)
    q_d = nc.declare_dram_parameter("q", [K7, 3 * A], f32, isOutput=False)
    out_d = nc.declare_dram_parameter("out", [I, A], f32, isOutput=True)

    with tile.TileContext(nc) as tc:
        with (
            tc.tile_pool(name="const", bufs=1) as const,
            tc.tile_pool(name="zp", bufs=2, space=PSUM) as zp,
            tc.tile_pool(name="radp", bufs=2, space=PSUM) as radp,
            tc.tile_pool(name="scr", bufs=2, space=PSUM) as scr,
            tc.tile_pool(name="ep", bufs=3) as ep,
            tc.tile_pool(name="angp", bufs=4) as angp,
            tc.tile_pool(name="mid", bufs=3) as mid,
            tc.tile_pool(name="op", bufs=3) as op,
            tc.tile_pool(name="tp", bufs=3) as tp,
        ):
            w1_sb = const.tile([K7, RT * 128], f32r)
            s_sb = const.tile([128, RT * 128], f32r)
            q_sb = const.tile([K7, 3 * A], f32r)
            r_sb = const.tile([K7, I], f32r)
            w1_st = const.tile([K7, RT * 128], f32)
            s_st = const.tile([128, RT * 128], f32)
            q_st = const.tile([K7, 3 * A], f32)
            r_st = const.tile([K7, I], f32)
            ident = const.tile([128, 128], f32)

            nc.sync.dma_start(w1_st[:], w1_d[:])
            nc.sync.dma_start(s_st[:], s_d[:])
            nc.sync.dma_start(q_st[:], q_d[:])
            masks.make_identity(nc, ident[:])
            # engine-round constants to f32r (walrus requires rounded operands)
            nc.gpsimd.tensor_copy(w1_sb[:], w1_st[:])
            nc.gpsimd.tensor_copy(s_sb[:], s_st[:])
            nc.gpsimd.tensor_copy(q_sb[:], q_st[:])

            # basis rows: 0..2 = squares, 3..5 = linear, 6 = ones
            nc.gpsimd.memset(r_st[:], 1.0)
            for c in range(3):
                col = pos_d[:, c:c + 1].rearrange("i one -> one i")
                nc.sync.dma_start(r_st[c:c + 1, :], col)
                nc.sync.dma_start(r_st[3 + c:4 + c, :], col)
            nc.gpsimd.tensor_copy(r_sb[:], r_st[:])
            nc.vector.tensor_mul(r_sb[0:3, :], r_st[0:3, :], r_st[0:3, :])

            def mm(out_ap, lhs_ap, rhs_ap, start=True, stop=True):
                nc.tensor.matmul(out_ap, lhs_ap, rhs_ap, start=start, stop=stop)

            for it in range(NIT):
                i0 = it * ITILE
                ri = r_sb[:, i0:i0 + ITILE]

                # ---- angular: p_c = Q_c^T R, ang = px*py*pz ----
                ang = []
                for at in range(2):
                    def qs(c):
                        a0 = c * A + at * 128
                        return q_sb[:, a0:a0 + 128]
                    px = scr.tile([128, ITILE], f32, tag="scr")
                    mm(px[:], qs(0), ri)
                    py = scr.tile([128, ITILE], f32, tag="scr")
                    mm(py[:], qs(1), ri)
                    pxs = mid.tile([128, ITILE], f32, tag="pxs")
                    nc.vector.tensor_copy(pxs[:], px[:])
                    t1 = mid.tile([128, ITILE], f32, tag="t1")
                    nc.vector.tensor_mul(t1[:], pxs[:], py[:])
                    pz = scr.tile([128, ITILE], f32, tag="scr")
                    mm(pz[:], qs(2), ri)
                    a_sb = angp.tile([128, ITILE], f32, tag="ang")
                    nc.vector.tensor_mul(a_sb[:], t1[:], pz[:])
                    ang.append(a_sb)

                # ---- radial: z = W1^T R (pairs of r-tiles), E=exp(z), rad = S^T E
                rad = []
                for _ in range(2):
                    rad_t = radp.tile([128, ITILE], f32, tag="rad")
                    rad.append(rad_t)
                for pair in range(RT // 2):
                    rts = (2 * pair, 2 * pair + 1)
                    at = 0 if rts[0] < 6 else 1
                    z2 = zp.tile([128, 2 * ITILE], f32, tag="z")
                    for j, rt in enumerate(rts):
                        mm(z2[:, j * ITILE:(j + 1) * ITILE],
                           w1_sb[:, rt * 128:(rt + 1) * 128], ri)
                    e2 = ep.tile([128, 2 * ITILE], f32r, tag="e")
                    nc.scalar.activation(e2[:], z2[:], EXP)
                    for j, rt in enumerate(rts):
                        mm(rad[at][:], s_sb[:, rt * 128:(rt + 1) * 128],
                           e2[:, j * ITILE:(j + 1) * ITILE],
                           start=(rt % 6 == 0), stop=(rt % 6 == 5))

                # ---- final: out = ang * rad, transpose, DMA out ----
                osb = []
                for at in range(2):
                    o = op.tile([128, ITILE], f32, tag="o")
                    nc.vector.tensor_mul(o[:], ang[at][:], rad[at][:])
                    osb.append(o)
                for blk in range(ITILE // 128):
                    tps = scr.tile([128, 2 * 128], f32, tag="scr")
                    for at in range(2):
                        nc.tensor.transpose(
                            tps[:, at * 128:(at + 1) * 128],
                            osb[at][:, blk * 128:(blk + 1) * 128], ident[:])
                    t_sb = tp.tile([128, 2 * 128], f32, tag="tsb")
                    nc.any.tensor_copy(t_sb[:], tps[:])
                    ib = i0 + blk * 128
                    nc.sync.dma_start(out_d[ib:ib + 128, :], t_sb[:])

    nc.compile()
    return nc


def _consts(centers, exps, coeffs, powers):
    al = exps.astype(np.float64)
    c = coeffs.astype(np.float64)
    cen = centers.astype(np.float64)
    cc = (cen ** 2).sum(-1)
    absc = np.abs(c)
    lnc = np.where(absc > 0, np.log(np.where(absc > 0, absc, 1.0)), -1e30)
    sgn = np.sign(c)

    alf = al.reshape(-1)  # row index r = a*P + p
    w1 = np.zeros((K7, A * P))
    w1[0] = w1[1] = w1[2] = -alf
    for cd in range(3):
        w1[3 + cd] = 2.0 * alf * np.repeat(cen[:, cd], P)
    w1[6] = -alf * np.repeat(cc, P) + lnc.reshape(-1)

    s = np.zeros((RT, 128, 128))
    r = np.arange(A * P)
    t_of_r = r // 128
    m_of_r = (r // P) - np.where(t_of_r < RT // 2, 0, 128)
    s[t_of_r, r % 128, m_of_r] = sgn.reshape(-1)

    q = np.zeros((3, K7, A))
    for cd in range(3):
        l = powers[:, cd].astype(np.int64)
        ccd = cen[:, cd]
        q[cd, cd] = (l == 2) * 1.0
        q[cd, 3 + cd] = (l == 1) * 1.0 + (l == 2) * (-2.0 * ccd)
        q[cd, 6] = (l == 0) * 1.0 + (l == 1) * (-ccd) + (l == 2) * (ccd ** 2)

    s2 = np.ascontiguousarray(s.transpose(1, 0, 2).reshape(128, RT * 128))
    q2 = np.ascontiguousarray(q.transpose(1, 0, 2).reshape(K7, 3 * A))
    return (w1.astype(np.float32), s2.astype(np.float32), q2.astype(np.float32))


LAST_RESULT = None


def kernel(pos, centers, exps, coeffs, powers):
    global LAST_RESULT
    from concourse.bass_utils import run_bass_kernel_spmd

    pos = np.asarray(pos, dtype=np.float32)
    centers = np.asarray(centers, dtype=np.float32)
    exps = np.asarray(exps, dtype=np.float32)
    coeffs = np.asarray(coeffs, dtype=np.float32)
    powers = np.asarray(powers)

    if "nc" not in _CACHE:
        _CACHE["nc"] = _build_nc()
    nc = _CACHE["nc"]

    w1, s, q = _consts(centers, exps, coeffs, powers)
    in_maps = []
    for ci in range(NCORES):
        shard = np.ascontiguousarray(
            pos[ci * BS:(ci + 1) * BS].reshape(I, 3))
        in_maps.append({"pos": shard, "w1": w1, "s": s, "q": q})

    res = run_bass_kernel_spmd(nc, in_maps, core_ids=list(range(NCORES)))
    LAST_RESULT = res
    out = np.concatenate(
        [res.results[ci]["out"].reshape(BS, NEL, A) for ci in range(NCORES)],
        axis=0)
    return out


# revision 7
# speedup vs baseline: 1.4765x; 1.4765x over previous
"""AO layer kernel for Trainium2 (8 NeuronCores, data-parallel over walkers).

Math: out[b,n,a] = ang(a, r) * rad(a, r),  r = pos[b,n] - centers[a]
  rad = sum_p coeffs[a,p] * exp(-exps[a,p] * |r|^2)
  ang = prod_c r_c^powers[a,c],  powers in {0,1,2}

Device formulation (per core, i = flattened (b,n) walker-electron index):
  Basis R[7, i] = {x^2, y^2, z^2, x, y, z, 1} of pos (squares built on device).
  z[(a,p), i]   = W1[7,(a,p)]^T R          (PE matmul; -alpha*r2 + ln|c| folded)
  E = exp(z)                               (ScalarE)
  rad[a, i]     = S[(a,p),a]^T E           (PE matmul with +-1 sign matrix)
  p_c[a, i]     = Q_c[7,a]^T R             (PE matmuls; per-axis angular polys)
  out[a, i]     = px*py*pz*rad             (VectorE)
  out[i, a]     = PE transpose, DMA to DRAM.

All matmuls run in float32r (TensorE full rate). Operands are pre-rounded
to f32r's 11-bit mantissa on the host (bitwise identical to what the
engines' own f32r rounding would produce), so matmul products are exact.
"""

import numpy as np

B, NEL, A, P = 512, 32, 256, 6
NCORES = 8
BS = B // NCORES          # 64 walkers per core
I = BS * NEL              # 2048 (b,n) pairs per core
ITILE = 512
NIT = I // ITILE          # 4 i-tiles
RT = (A * P) // 128       # 12 r-tiles of 128 (a,p) rows
K7 = 7

_CACHE = {}


def _r11(v):
    """Round f32 array to f32r (11 explicit mantissa bits), RNE."""
    u = np.ascontiguousarray(np.asarray(v, np.float32)).view(np.uint32)
    low = u & np.uint32(0xFFF)
    add = np.where((low > 0x800) | ((low == 0x800) & (((u >> 12) & 1) == 1)),
                   0x1000, 0).astype(np.uint32)
    return ((u & ~np.uint32(0xFFF)) + add).view(np.float32)


def _build_nc():
    import concourse.bass as bass
    import concourse.bacc as bacc
    import concourse.tile as tile
    import concourse.mybir as mybir
    from concourse import masks

    f32 = mybir.dt.float32
    f32r = mybir.dt.float32r
    EXP = mybir.ActivationFunctionType.Exp
    PSUM = bass.MemorySpace.PSUM

    nc = bacc.Bacc("TRN2", target_bir_lowering=False, debug=False,
                   num_devices=NCORES)

    pos_d = nc.declare_dram_parameter("posT", [4, I], f32r, isOutput=False)
    w1_d = nc.declare_dram_parameter("w1", [K7, RT * 128], f32r, isOutput=False)
    s_d = nc.declare_dram_parameter("s", [128, RT * 128], f32r, isOutput=False)
    q_d = nc.declare_dram_parameter("q", [K7, 3 * A], f32r, isOutput=False)
    out_d = nc.declare_dram_parameter("out", [I, A], f32, isOutput=True)

    with tile.TileContext(nc) as tc:
        with (
            tc.tile_pool(name="const", bufs=1) as const,
            tc.tile_pool(name="zp", bufs=2, space=PSUM) as zp,
            tc.tile_pool(name="radp", bufs=2, space=PSUM) as radp,
            tc.tile_pool(name="scr", bufs=2, space=PSUM) as scr,
            tc.tile_pool(name="ep", bufs=3) as ep,
            tc.tile_pool(name="angp", bufs=4) as angp,
            tc.tile_pool(name="mid", bufs=3) as mid,
            tc.tile_pool(name="op", bufs=3) as op,
            tc.tile_pool(name="tp", bufs=3) as tp,
        ):
            w1_sb = const.tile([K7, RT * 128], f32r)
            s_sb = const.tile([128, RT * 128], f32r)
            q_sb = const.tile([K7, 3 * A], f32r)
            r_sb = const.tile([K7, I], f32r)
            sq_src = const.tile([3, I], f32r)
            ident = const.tile([128, 128], f32)

            nc.sync.dma_start(w1_sb[:], w1_d[:])
            nc.sync.dma_start(s_sb[:], s_d[:])
            nc.sync.dma_start(q_sb[:], q_d[:])
            masks.make_identity(nc, ident[:])

            # basis rows: 0..2 = squares (device), 3..6 = x,y,z,1 (direct)
            nc.sync.dma_start(r_sb[3:7, :], pos_d[:])
            nc.sync.dma_start(sq_src[:], pos_d[0:3, :])
            nc.vector.tensor_mul(r_sb[0:3, :], sq_src[:], sq_src[:])

            def mm(out_ap, lhs_ap, rhs_ap, start=True, stop=True):
                nc.tensor.matmul(out_ap, lhs_ap, rhs_ap, start=start, stop=stop)

            for it in range(NIT):
                i0 = it * ITILE
                ri = r_sb[:, i0:i0 + ITILE]

                # ---- angular: p_c = Q_c^T R, ang = px*py*pz ----
                ang = []
                for at in range(2):
                    def qs(c):
                        a0 = c * A + at * 128
                        return q_sb[:, a0:a0 + 128]
                    px = scr.tile([128, ITILE], f32, tag="scr")
                    mm(px[:], qs(0), ri)
                    py = scr.tile([128, ITILE], f32, tag="scr")
                    mm(py[:], qs(1), ri)
                    pxs = mid.tile([128, ITILE], f32, tag="pxs")
                    nc.vector.tensor_copy(pxs[:], px[:])
                    t1 = mid.tile([128, ITILE], f32, tag="t1")
                    nc.vector.tensor_mul(t1[:], pxs[:], py[:])
                    pz = scr.tile([128, ITILE], f32, tag="scr")
                    mm(pz[:], qs(2), ri)
                    a_sb = angp.tile([128, ITILE], f32, tag="ang")
                    nc.vector.tensor_mul(a_sb[:], t1[:], pz[:])
                    ang.append(a_sb)

                # ---- radial: z = W1^T R (pairs of r-tiles), E=exp(z), rad = S^T E
                rad = []
                for _ in range(2):
                    rad_t = radp.tile([128, ITILE], f32, tag="rad")
                    rad.append(rad_t)
                for pair in range(RT // 2):
                    rts = (2 * pair, 2 * pair + 1)
                    at = 0 if rts[0] < 6 else 1
                    z2 = zp.tile([128, 2 * ITILE], f32, tag="z")
                    for j, rt in enumerate(rts):
                        mm(z2[:, j * ITILE:(j + 1) * ITILE],
                           w1_sb[:, rt * 128:(rt + 1) * 128], ri)
                    e2 = ep.tile([128, 2 * ITILE], f32r, tag="e")
                    nc.scalar.activation(e2[:], z2[:], EXP)
                    for j, rt in enumerate(rts):
                        mm(rad[at][:], s_sb[:, rt * 128:(rt + 1) * 128],
                           e2[:, j * ITILE:(j + 1) * ITILE],
                           start=(rt % 6 == 0), stop=(rt % 6 == 5))

                # ---- final: out = ang * rad, transpose, DMA out ----
                osb = []
                for at in range(2):
                    o = op.tile([128, ITILE], f32, tag="o")
                    nc.vector.tensor_mul(o[:], ang[at][:], rad[at][:])
                    osb.append(o)
                for blk in range(ITILE // 128):
                    tps = scr.tile([128, 2 * 128], f32, tag="scr")
                    for at in range(2):
                        nc.tensor.transpose(
                            tps[:, at * 128:(at + 1) * 128],
                            osb[at][:, blk * 128:(blk + 1) * 128], ident[:])
                    t_sb = tp.tile([128, 2 * 128], f32, tag="tsb")
                    nc.any.tensor_copy(t_sb[:], tps[:])
                    ib = i0 + blk * 128
                    nc.sync.dma_start(out_d[ib:ib + 128, :], t_sb[:])

    nc.compile()
    return nc


def _consts(centers, exps, coeffs, powers):
    al = exps.astype(np.float64)
    c = coeffs.astype(np.float64)
    cen = centers.astype(np.float64)
    cc = (cen ** 2).sum(-1)
    absc = np.abs(c)
    lnc = np.where(absc > 0, np.log(np.where(absc > 0, absc, 1.0)), -1e30)
    sgn = np.sign(c)

    alf = al.reshape(-1)  # row index r = a*P + p
    w1 = np.zeros((K7, A * P))
    w1[0] = w1[1] = w1[2] = -alf
    for cd in range(3):
        w1[3 + cd] = 2.0 * alf * np.repeat(cen[:, cd], P)
    w1[6] = -alf * np.repeat(cc, P) + lnc.reshape(-1)

    s = np.zeros((RT, 128, 128))
    r = np.arange(A * P)
    t_of_r = r // 128
    m_of_r = (r // P) - np.where(t_of_r < RT // 2, 0, 128)
    s[t_of_r, r % 128, m_of_r] = sgn.reshape(-1)

    q = np.zeros((3, K7, A))
    for cd in range(3):
        l = powers[:, cd].astype(np.int64)
        ccd = cen[:, cd]
        q[cd, cd] = (l == 2) * 1.0
        q[cd, 3 + cd] = (l == 1) * 1.0 + (l == 2) * (-2.0 * ccd)
        q[cd, 6] = (l == 0) * 1.0 + (l == 1) * (-ccd) + (l == 2) * (ccd ** 2)

    s2 = np.ascontiguousarray(s.transpose(1, 0, 2).reshape(128, RT * 128))
    q2 = np.ascontiguousarray(q.transpose(1, 0, 2).reshape(K7, 3 * A))
    return (_r11(w1), _r11(s2), _r11(q2))


LAST_RESULT = None


def kernel(pos, centers, exps, coeffs, powers):
    global LAST_RESULT
    from concourse.bass_utils import run_bass_kernel_spmd

    pos = np.asarray(pos, dtype=np.float32)
    centers = np.asarray(centers, dtype=np.float32)
    exps = np.asarray(exps, dtype=np.float32)
    coeffs = np.asarray(coeffs, dtype=np.float32)
    powers = np.asarray(powers)

    if "nc" not in _CACHE:
        _CACHE["nc"] = _build_nc()
    nc = _CACHE["nc"]

    w1, s, q = _consts(centers, exps, coeffs, powers)
    in_maps = []
    for ci in range(NCORES):
        shard = pos[ci * BS:(ci + 1) * BS].reshape(I, 3)
        post = np.concatenate(
            [shard.T, np.ones((1, I), np.float32)], axis=0)
        in_maps.append({"posT": _r11(post), "w1": w1, "s": s, "q": q})

    res = run_bass_kernel_spmd(nc, in_maps, core_ids=list(range(NCORES)))
    LAST_RESULT = res
    out = np.concatenate(
        [res.results[ci]["out"].reshape(BS, NEL, A) for ci in range(NCORES)],
        axis=0)
    return out
